# revision 5
# baseline (speedup 1.0000x reference)
"""GPTQ 4-bit dequant + linear (x @ W.T + bias) on 8 Trainium2 NeuronCores.

Problem shapes (hardcoded):
  x       [4, 2048, 4096] f32   -> host-tiled to bf16 [64, 128, 32, 128]
  qweight [16384, 512]    i32   (8x 4-bit nibbles per int32 along K)
  qzeros  [16384, 4]      i32
  scales  [16384, 32]     f32
  bias    [16384]         f32
  out     [4, 2048, 16384] f32

Sharding: column-parallel over out_features. Each of the 8 cores gets a
2048-row slab of qweight/qzeros/scales/bias; x replicated; outputs are
concatenated on the host along the feature axis.

Host-side prep (layout only, no compute): x is cast to bf16 and permuted
to [chunk, kk, c, t] tiles so each 128-token chunk is ONE contiguous DMA
directly into the [128 kk, 32 c, 128 t] transposed SBUF layout the PE
needs -- no on-device x transposes at all. qweight is viewed as int16 so
nibble extraction runs at the DVE's 2x 16-bit rate.

Per-core kernel:
  Phase A: dequantize the int4 slab to bf16 W.T resident in SBUF
           ([128 kk, 32 c, 2048 n]). Nibble extract on DVE (int16),
           per-group (q-z)*s split across DVE / ACT / GPSIMD, one
           batched xbar transpose per 128-row n-chunk (sync queue is
           dedicated to these). qw/qz/sc loads go on the gpsimd SWDGE
           queue; x tile loads and output stores on the scalar HWDGE.
  Phase B: per 128-token chunk: one DMA into xT, then per 512-col
           n-block: 32 PE matmuls accumulating one PSUM bank, DVE
           PSUM+bias -> SBUF add, store on the scalar HWDGE ring.
  The first npro token chunks are processed n-block-major, interleaved
  with Phase A emission, so their matmuls execute inside Phase A's
  otherwise-idle PE windows (each 512-col n-block becomes available
  after 4 Phase-A chunks).
"""
import sys

for _p in ("/opt/trn_rl_repo", "/root/.axon_site/_ro/trn_rl_repo"):
    if _p not in sys.path:
        sys.path.append(_p)

import numpy as np
import ml_dtypes
import concourse.bass as bass
import concourse.mybir as mybir
from concourse import tile, bacc
from concourse.bass_utils import run_bass_kernel_spmd

BF16 = mybir.dt.bfloat16
F32 = mybir.dt.float32
I32 = mybir.dt.int32
I16 = mybir.dt.int16

B, S, K, N = 4, 2048, 4096, 16384
T = B * S                      # 8192 tokens
NCORES = 8
NS = N // NCORES               # 2048 out features per core
PACK = 8
GS = 128                       # quant group size
G = K // GS                    # 32 groups == 32 k-chunks
TCH = 128                      # tokens per chunk
KC = K // 128                  # 32 k-chunks
MMN = 512                      # matmul moving free dim (one PSUM bank of f32)
NBLK = NS // MMN               # 4
NCH = NS // 128                # 16 weight n-chunks
HALF = K // 2                  # dequant processed in 2 half-chunks
K16 = K // PACK * 2            # 1024 int16 words per weight row
NPRO = 4                       # prologue token chunks overlapped with Phase A

_LSR = mybir.AluOpType.logical_shift_right
_AND = mybir.AluOpType.bitwise_and
_SUB = mybir.AluOpType.subtract
_MUL = mybir.AluOpType.mult
_ADD = mybir.AluOpType.add
IDENT = mybir.ActivationFunctionType.Identity

# engine per quant group (32 entries): v=DVE, a=ACT, g=GPSIMD
ENG32 = list("vagvavavavavavaa" "vagvavavavavavag")
assert len(ENG32) == 32


def build(t_total: int = T):
    nt = t_total // TCH
    nc = bacc.Bacc("TRN2", target_bir_lowering=False, debug=False)
    xt_d = nc.dram_tensor("xt", [nt, 128, KC, TCH], BF16, kind="ExternalInput")
    qw_d = nc.dram_tensor("qw", [NS, K16], I16, kind="ExternalInput")
    qz_d = nc.dram_tensor("qz", [NS, G // PACK], I32, kind="ExternalInput")
    sc_d = nc.dram_tensor("sc", [NS, G], F32, kind="ExternalInput")
    b_d = nc.dram_tensor("b", [NS], F32, kind="ExternalInput")
    out_d = nc.dram_tensor("out", [t_total, NS], F32, kind="ExternalOutput")

    with tile.TileContext(nc) as tc:
        with (
            tc.tile_pool(name="wtp", bufs=1) as wtpool,
            tc.tile_pool(name="consts", bufs=1) as cpool,
            tc.tile_pool(name="aload", bufs=2) as apool,
            tc.tile_pool(name="anib", bufs=2) as nibpool,
            tc.tile_pool(name="awch", bufs=2) as wchpool,
            tc.tile_pool(name="bxt", bufs=NPRO + 1) as bxtpool,
            tc.tile_pool(name="bout", bufs=2) as bopool,
            tc.tile_pool(name="ps", bufs=8, space=bass.MemorySpace.PSUM) as pspool,
        ):
            # persistent dequantized W.T: [128 kk, 32 c, 2048 n] bf16
            wT = wtpool.tile([128, KC, NS], BF16)

            # helpers -------------------------------------------------
            def mm_block(ps_t, xT_t, nb):
                for c in range(KC):
                    nc.tensor.matmul(
                        ps_t[:], xT_t[:, c, :],
                        wT[:, c, nb * MMN:(nb + 1) * MMN],
                        start=(c == 0), stop=(c == KC - 1))

            def drain_store(ps_t, t0, nb):
                o_t = bopool.tile([128, MMN], F32, name="o_nb", tag="o_nb")
                nc.vector.tensor_tensor(
                    out=o_t[:], in0=ps_t[:],
                    in1=bias_t[:, nb * MMN:(nb + 1) * MMN], op=_ADD)
                nc.scalar.dma_start(
                    out_d[t0:t0 + TCH, nb * MMN:(nb + 1) * MMN], o_t[:])

            # prologue: stage the first NPRO token chunks' xT tiles early;
            # their matmuls run inside Phase A's idle PE windows
            npro = min(NPRO, nt)
            pro_xT = []
            for ti in range(npro):
                xT_t = bxtpool.tile([128, KC, TCH], BF16)
                nc.scalar.dma_start(xT_t[:], xt_d[ti])
                pro_xT.append(xT_t)

            # bias broadcast to all 128 partitions: [128, 2048] bf16
            # (cast+broadcast during SWDGE DMA; added in f32 at PSUM drain)
            bias_t = cpool.tile([128, NS], BF16)
            b_row = b_d[:].rearrange("(o n) -> o n", o=1)
            b_bcast = bass.AP(tensor=b_row.tensor, offset=b_row.offset,
                              ap=[[0, 128], b_row.ap[1]])
            nc.gpsimd.dma_start(out=bias_t[:], in_=b_bcast)

            # ---- Phase A: dequantize weight slab, n-chunks of 128 rows
            for j in range(NCH):
                n0 = j * 128
                qw_t = apool.tile([128, K16], I16)
                nc.gpsimd.dma_start(qw_t[:], qw_d[n0:n0 + 128, :])
                qz_t = apool.tile([128, G // PACK], I32)
                nc.gpsimd.dma_start(qz_t[:], qz_d[n0:n0 + 128, :])
                sc_t = apool.tile([128, G], F32)
                nc.gpsimd.dma_start(sc_t[:], sc_d[n0:n0 + 128, :])

                zi_t = apool.tile([128, G], I32)
                for i in range(PACK):
                    nc.vector.tensor_scalar(
                        out=zi_t[:, i::PACK], in0=qz_t[:],
                        scalar1=4 * i, scalar2=0xF, op0=_LSR, op1=_AND)
                z_t = apool.tile([128, G], F32)
                nc.vector.tensor_copy(z_t[:], zi_t[:])
                # zs = -z * s  (ACT bias operand)
                zs_t = apool.tile([128, G], F32)
                nc.vector.scalar_tensor_tensor(
                    out=zs_t[:], in0=z_t[:], scalar=-1.0, in1=sc_t[:],
                    op0=_MUL, op1=_MUL)

                w_t = wchpool.tile([128, K], BF16)
                for h in range(2):
                    # int16 view: word w of half h = elems 512h+2w (lo),
                    # 512h+2w+1 (hi); nibble i of word w -> k = 8w+i
                    nib_t = nibpool.tile([128, HALF], I16)
                    for i in range(PACK):
                        src0 = 512 * h + (1 if i >= 4 else 0)
                        nc.vector.tensor_scalar(
                            out=nib_t[:, i::PACK],
                            in0=qw_t[:, src0:src0 + 511:2],
                            scalar1=4 * (i % 4), scalar2=0xF,
                            op0=_LSR, op1=_AND)
                    for gh in range(G // 2):
                        g = h * (G // 2) + gh
                        eng = ENG32[g]
                        if eng == 'a':
                            # ACT: out = nib * s + (-z*s)
                            nc.scalar.activation(
                                w_t[:, g * GS:(g + 1) * GS],
                                nib_t[:, gh * GS:(gh + 1) * GS],
                                IDENT, bias=zs_t[:, g:g + 1],
                                scale=sc_t[:, g:g + 1])
                        else:
                            # DVE / GPSIMD: out = (nib - z) * s
                            e = nc.vector if eng == 'v' else nc.gpsimd
                            e.tensor_scalar(
                                out=w_t[:, g * GS:(g + 1) * GS],
                                in0=nib_t[:, gh * GS:(gh + 1) * GS],
                                scalar1=z_t[:, g:g + 1], scalar2=sc_t[:, g:g + 1],
                                op0=_SUB, op1=_MUL)

                # one batched xbar transpose: w_t [128 n, 4096 k]
                #   -> wT[:, :, n0:n0+128]  ([128 kk, 32 c, 128 n])
                nc.sync.dma_start_transpose(wT[:, :, n0:n0 + 128], w_t[:])

                # prologue n-block scheduling: after window w = j//4 of 4
                # n-chunks is emitted, queue its matmuls + drains
                if j % 4 == 3:
                    w = j // 4
                    for ti in range(npro):
                        ps_t = pspool.tile([128, MMN], F32,
                                           name="psnb", tag="psnb")
                        mm_block(ps_t, pro_xT[ti], w)
                        drain_store(ps_t, ti * TCH, w)

            # ---- Phase B: stream remaining tokens
            for ti in range(npro, nt):
                t0 = ti * TCH
                xT_t = bxtpool.tile([128, KC, TCH], BF16)
                nc.scalar.dma_start(xT_t[:], xt_d[ti])
                for nb in range(NBLK):
                    ps_t = pspool.tile([128, MMN], F32, name="psnb", tag="psnb")
                    mm_block(ps_t, xT_t, nb)
                    drain_store(ps_t, t0, nb)

    nc.compile()
    return nc


_nc_cache = {}


def _get_nc(t_total: int = T):
    if t_total not in _nc_cache:
        _nc_cache[t_total] = build(t_total)
    return _nc_cache[t_total]


def _tile_x(x, t_total):
    # [T, K] f32 -> bf16 [nt, 128 kk, 32 c, 128 t] so one chunk is one
    # contiguous DMA into the transposed SBUF layout
    nt = t_total // TCH
    xf = x.reshape(-1, K)[:t_total].astype(ml_dtypes.bfloat16)
    xt = xf.reshape(nt, TCH, KC, 128).transpose(0, 3, 2, 1)
    return np.ascontiguousarray(xt)


def kernel(x, qweight, qzeros, scales, bias, trace=False, t_total=T):
    xt = _tile_x(np.asarray(x, dtype=np.float32), t_total)
    qw16 = np.ascontiguousarray(qweight).view(np.int16)
    in_maps = []
    for c in range(NCORES):
        sl = slice(c * NS, (c + 1) * NS)
        in_maps.append({
            "xt": xt,
            "qw": np.ascontiguousarray(qw16[sl]),
            "qz": np.ascontiguousarray(qzeros[sl]),
            "sc": np.ascontiguousarray(scales[sl]),
            "b": np.ascontiguousarray(bias[sl]),
        })
    nc = _get_nc(t_total)
    res = run_bass_kernel_spmd(nc, in_maps, core_ids=list(range(NCORES)),
                               trace=trace)
    out = np.concatenate([r["out"] for r in res.results], axis=1)
    if t_total == T:
        out = out.reshape(B, S, N)
    out = out.astype(np.float32, copy=False)
    if trace:
        return out, res
    return out


# revision 8
# speedup vs baseline: 1.1103x; 1.1103x over previous
"""GPTQ 4-bit dequant + linear (x @ W.T + bias) on 8 Trainium2 NeuronCores.

Problem shapes (hardcoded):
  x       [4, 2048, 4096] f32   -> host-tiled to bf16 [64, 128, 32, 128]
  qweight [16384, 512]    i32   (8x 4-bit nibbles per int32 along K)
  qzeros  [16384, 4]      i32
  scales  [16384, 32]     f32
  bias    [16384]         f32
  out     [4, 2048, 16384] f32

Sharding: column-parallel over out_features. Each of the 8 cores gets a
2048-row slab of qweight/qzeros/scales/bias; x replicated; outputs are
concatenated on the host along the feature axis.

Host-side prep (layout only): x is scaled by 2^4, cast to bf16 and
permuted to [chunk, kk, c, t] tiles so each 128-token chunk is ONE
contiguous DMA directly into the [128 kk, 32 c, 128 t] transposed SBUF
layout the PE needs -- no on-device x transposes.

Mixed precision: the last NFP8 of the 32 k-chunks run as fp8e4
DoubleRow matmuls (2 k-chunks per MM at ~1.4x bf16 throughput); the
rest stay bf16. To make e4m3's range work, the whole kernel computes
at scale 2^12: x is pre-scaled by 2^4 (host), weights dequantize to
w*2^8 (device), and the PSUM drain rescales by 2^-12 in the same fused
DVE op that adds the bias. Error budget: fp8 on 6/32 of K measures
rel_err ~1.75e-2 vs the 2e-2 gate.

Per-core kernel:
  Phase A: dequantize the int4 slab to (w*256) resident in SBUF:
           bf16 [128 kk, 26 c, 2048 n] + fp8e4 [128 kk, 6 c, 2048 n].
           Nibble extract on DVE (int32), per-group (q-z)*s split
           across ACT/DVE, one batched xbar transpose per 128-row
           n-chunk (sync queue dedicated to transposes; qw/qz/sc loads
           on the gpsimd SWDGE queue; x loads + stores on scalar).
  Phase B: per 128-token chunk: one DMA into xT, ACT cast of the fp8
           slice, then per 512-col n-block: 26 bf16 + 3 DoubleRow PE
           matmuls accumulating one PSUM bank, fused DVE
           (PSUM*2^-12)+bias -> SBUF, store on the scalar HWDGE ring.
  The first NPRO token chunks are processed n-block-major, interleaved
  with Phase A emission, so their matmuls execute inside Phase A's
  otherwise-idle PE windows (each 512-col n-block becomes available
  after 4 Phase-A chunks).
"""
import sys

for _p in ("/opt/trn_rl_repo", "/root/.axon_site/_ro/trn_rl_repo"):
    if _p not in sys.path:
        sys.path.append(_p)

import numpy as np
import ml_dtypes
import concourse.bass as bass
import concourse.mybir as mybir
from concourse import tile, bacc
from concourse.bass_utils import run_bass_kernel_spmd

BF16 = mybir.dt.bfloat16
F32 = mybir.dt.float32
I32 = mybir.dt.int32
FP8 = mybir.dt.float8e4

B, S, K, N = 4, 2048, 4096, 16384
T = B * S                      # 8192 tokens
NCORES = 8
NS = N // NCORES               # 2048 out features per core
PACK = 8
GS = 128                       # quant group size
G = K // GS                    # 32 groups == 32 k-chunks
TCH = 128                      # tokens per chunk
KC = K // 128                  # 32 k-chunks
MMN = 512                      # matmul moving free dim (one PSUM bank of f32)
NBLK = NS // MMN               # 4
NCH = NS // 128                # 16 weight n-chunks
HALF = K // 2                  # dequant processed in 2 half-chunks
NPRO = 4                       # prologue token chunks overlapped with Phase A
NFP8 = 6                       # trailing k-chunks on the fp8 DoubleRow path
KCB = KC - NFP8                # leading bf16 k-chunks
XSC = 16.0                     # x pre-scale (2^4, host side)
WSC = 256.0                    # w dequant scale (2^8, device side)
OSC = 1.0 / (XSC * WSC)        # drain rescale 2^-12

_LSR = mybir.AluOpType.logical_shift_right
_AND = mybir.AluOpType.bitwise_and
_SUB = mybir.AluOpType.subtract
_MUL = mybir.AluOpType.mult
_ADD = mybir.AluOpType.add
IDENT = mybir.ActivationFunctionType.Identity
DROW = mybir.MatmulPerfMode.DoubleRow

# engine per quant group: 20 ACT / 12 DVE
ENG32 = ['a' if g % 8 < 5 else 'v' for g in range(32)]


def build(t_total: int = T):
    nt = t_total // TCH
    nc = bacc.Bacc("TRN2", target_bir_lowering=False, debug=False)
    xt_d = nc.dram_tensor("xt", [nt, 128, KC, TCH], BF16, kind="ExternalInput")
    qw_d = nc.dram_tensor("qw", [NS, K // PACK], I32, kind="ExternalInput")
    qz_d = nc.dram_tensor("qz", [NS, G // PACK], I32, kind="ExternalInput")
    sc_d = nc.dram_tensor("sc", [NS, G], F32, kind="ExternalInput")
    b_d = nc.dram_tensor("b", [NS], F32, kind="ExternalInput")
    out_d = nc.dram_tensor("out", [t_total, NS], F32, kind="ExternalOutput")

    with tile.TileContext(nc) as tc:
        with (
            tc.tile_pool(name="wtp", bufs=1) as wtpool,
            tc.tile_pool(name="consts", bufs=1) as cpool,
            tc.tile_pool(name="aload", bufs=2) as apool,
            tc.tile_pool(name="anib", bufs=2) as nibpool,
            tc.tile_pool(name="awch", bufs=2) as wchpool,
            tc.tile_pool(name="awt8", bufs=2) as w8pool,
            tc.tile_pool(name="bxt", bufs=NPRO + 1) as bxtpool,
            tc.tile_pool(name="bxq", bufs=NPRO) as xqpool,
            tc.tile_pool(name="bout", bufs=2) as bopool,
            tc.tile_pool(name="ps", bufs=8, space=bass.MemorySpace.PSUM) as pspool,
        ):
            # persistent dequantized W.T * 256:
            #   bf16 [128 kk, 26 c, 2048 n] + fp8 [128 kk, 6 c, 2048 n]
            wT = wtpool.tile([128, KCB, NS], BF16)
            wT8 = wtpool.tile([128, NFP8, NS], FP8)

            # helpers -------------------------------------------------
            def mm_block(ps_t, xT_t, xq_t, nb):
                nsl = slice(nb * MMN, (nb + 1) * MMN)
                for c in range(KCB):
                    nc.tensor.matmul(
                        ps_t[:], xT_t[:, c, :], wT[:, c, nsl],
                        start=(c == 0), stop=False)
                for cp in range(0, NFP8, 2):
                    nc.tensor.matmul(
                        ps_t[:], xq_t[:, cp:cp + 2, :], wT8[:, cp:cp + 2, nsl],
                        start=False, stop=(cp == NFP8 - 2), perf_mode=DROW)

            def drain_store(ps_t, t0, nb):
                o_t = bopool.tile([128, MMN], F32, name="o_nb", tag="o_nb")
                # out = psum * 2^-12 + bias  (one fused DVE op)
                nc.vector.scalar_tensor_tensor(
                    out=o_t[:], in0=ps_t[:], scalar=OSC,
                    in1=bias_t[:, nb * MMN:(nb + 1) * MMN],
                    op0=_MUL, op1=_ADD)
                nc.scalar.dma_start(
                    out_d[t0:t0 + TCH, nb * MMN:(nb + 1) * MMN], o_t[:])

            def cast_xq(xT_t):
                # fp8 copy of the trailing NFP8 k-chunks (ACT, idle in B)
                xq_t = xqpool.tile([128, NFP8, TCH], FP8, name="xq", tag="xq")
                nc.scalar.copy(xq_t[:], xT_t[:, KCB:KC, :])
                return xq_t

            # prologue: stage the first NPRO token chunks' xT tiles early;
            # their matmuls run inside Phase A's idle PE windows
            npro = min(NPRO, nt)
            pro_xT = []
            pro_xq = []
            for ti in range(npro):
                xT_t = bxtpool.tile([128, KC, TCH], BF16)
                nc.scalar.dma_start(xT_t[:], xt_d[ti])
                pro_xT.append(xT_t)
            for ti in range(npro):
                pro_xq.append(cast_xq(pro_xT[ti]))

            # bias broadcast to all 128 partitions: [128, 2048] bf16
            bias_t = cpool.tile([128, NS], BF16)
            b_row = b_d[:].rearrange("(o n) -> o n", o=1)
            b_bcast = bass.AP(tensor=b_row.tensor, offset=b_row.offset,
                              ap=[[0, 128], b_row.ap[1]])
            nc.gpsimd.dma_start(out=bias_t[:], in_=b_bcast)

            # ---- Phase A: dequantize weight slab, n-chunks of 128 rows
            for j in range(NCH):
                n0 = j * 128
                qw_t = apool.tile([128, K // PACK], I32)
                nc.gpsimd.dma_start(qw_t[:], qw_d[n0:n0 + 128, :])
                qz_t = apool.tile([128, G // PACK], I32)
                nc.gpsimd.dma_start(qz_t[:], qz_d[n0:n0 + 128, :])
                sc_t = apool.tile([128, G], F32)
                nc.gpsimd.dma_start(sc_t[:], sc_d[n0:n0 + 128, :])

                zi_t = apool.tile([128, G], I32)
                for i in range(PACK):
                    nc.vector.tensor_scalar(
                        out=zi_t[:, i::PACK], in0=qz_t[:],
                        scalar1=4 * i, scalar2=0xF, op0=_LSR, op1=_AND)
                z_t = apool.tile([128, G], F32)
                nc.vector.tensor_copy(z_t[:], zi_t[:])
                # s256 = s * 256;  zs = -z * s * 256  (ACT scale/bias pair)
                s256_t = apool.tile([128, G], F32)
                nc.vector.tensor_scalar(
                    out=s256_t[:], in0=sc_t[:], scalar1=WSC, scalar2=0.0,
                    op0=_MUL, op1=_ADD)
                zs_t = apool.tile([128, G], F32)
                nc.vector.scalar_tensor_tensor(
                    out=zs_t[:], in0=z_t[:], scalar=-1.0, in1=s256_t[:],
                    op0=_MUL, op1=_MUL)

                w_t = wchpool.tile([128, K], BF16)
                for h in range(2):
                    w0 = h * (HALF // PACK)
                    nib_t = nibpool.tile([128, HALF], I32)
                    for i in range(PACK):
                        nc.vector.tensor_scalar(
                            out=nib_t[:, i::PACK],
                            in0=qw_t[:, w0:w0 + HALF // PACK],
                            scalar1=4 * i, scalar2=0xF, op0=_LSR, op1=_AND)
                    for gh in range(G // 2):
                        g = h * (G // 2) + gh
                        if ENG32[g] == 'a':
                            # ACT: out = nib * (s*256) + (-z*s*256)
                            nc.scalar.activation(
                                w_t[:, g * GS:(g + 1) * GS],
                                nib_t[:, gh * GS:(gh + 1) * GS],
                                IDENT, bias=zs_t[:, g:g + 1],
                                scale=s256_t[:, g:g + 1])
                        else:
                            # DVE: out = (nib - z) * (s*256)
                            nc.vector.tensor_scalar(
                                out=w_t[:, g * GS:(g + 1) * GS],
                                in0=nib_t[:, gh * GS:(gh + 1) * GS],
                                scalar1=z_t[:, g:g + 1],
                                scalar2=s256_t[:, g:g + 1],
                                op0=_SUB, op1=_MUL)

                # batched xbar transposes: w_t [128 n, 4096 k]
                #   bf16 chunks -> wT[:, :, n0:n0+128] ([128 kk, 26 c, 128 n])
                #   fp8 chunks: bf16 transpose to wtmp, then DVE cast
                nc.sync.dma_start_transpose(
                    wT[:, :, n0:n0 + 128], w_t[:, :KCB * 128])
                w8t_t = w8pool.tile([128, NFP8, 128], BF16)
                nc.sync.dma_start_transpose(w8t_t[:], w_t[:, KCB * 128:])
                nc.vector.tensor_copy(wT8[:, :, n0:n0 + 128], w8t_t[:])

                # prologue n-block scheduling: after window w = j//4 of 4
                # n-chunks is emitted, queue its matmuls + drains
                if j % 4 == 3:
                    w = j // 4
                    for ti in range(npro):
                        ps_t = pspool.tile([128, MMN], F32,
                                           name="psnb", tag="psnb")
                        mm_block(ps_t, pro_xT[ti], pro_xq[ti], w)
                        drain_store(ps_t, ti * TCH, w)

            # ---- Phase B: stream remaining tokens
            for ti in range(npro, nt):
                t0 = ti * TCH
                xT_t = bxtpool.tile([128, KC, TCH], BF16)
                nc.scalar.dma_start(xT_t[:], xt_d[ti])
                xq_t = cast_xq(xT_t)
                for nb in range(NBLK):
                    ps_t = pspool.tile([128, MMN], F32, name="psnb", tag="psnb")
                    mm_block(ps_t, xT_t, xq_t, nb)
                    drain_store(ps_t, t0, nb)

    nc.compile()
    return nc


_nc_cache = {}


def _get_nc(t_total: int = T):
    if t_total not in _nc_cache:
        _nc_cache[t_total] = build(t_total)
    return _nc_cache[t_total]


def _tile_x(x, t_total):
    # [T, K] f32 -> bf16 [nt, 128 kk, 32 c, 128 t] of x*2^4 so one chunk
    # is one contiguous DMA into the transposed SBUF layout
    nt = t_total // TCH
    xf = (x.reshape(-1, K)[:t_total] * np.float32(XSC)).astype(ml_dtypes.bfloat16)
    xt = xf.reshape(nt, TCH, KC, 128).transpose(0, 3, 2, 1)
    return np.ascontiguousarray(xt)


def kernel(x, qweight, qzeros, scales, bias, trace=False, t_total=T):
    xt = _tile_x(np.asarray(x, dtype=np.float32), t_total)
    in_maps = []
    for c in range(NCORES):
        sl = slice(c * NS, (c + 1) * NS)
        in_maps.append({
            "xt": xt,
            "qw": np.ascontiguousarray(qweight[sl]),
            "qz": np.ascontiguousarray(qzeros[sl]),
            "sc": np.ascontiguousarray(scales[sl]),
            "b": np.ascontiguousarray(bias[sl]),
        })
    nc = _get_nc(t_total)
    res = run_bass_kernel_spmd(nc, in_maps, core_ids=list(range(NCORES)),
                               trace=trace)
    out = np.concatenate([r["out"] for r in res.results], axis=1)
    if t_total == T:
        out = out.reshape(B, S, N)
    out = out.astype(np.float32, copy=False)
    if trace:
        return out, res
    return out
